# revision 9
# baseline (speedup 1.0000x reference)
"""Multi-head attention (B=2, S=2048, D=1024, H=16, d_k=64) on 8 TRN2 cores.

Sharding: core c = (batch b = c//4, head-group g = c%4); each core computes its
4 heads' attention and the partial output projection attn_g @ W_o_g^T; the host
sums the 4 per-batch partials (bf16 on device, f32 on host) and adds b_o.

Structure (v4; ~222us vs the 254us v1 head-sequential ACT-paced loop):
- Attention runs in q-blocks of 512 over head PAIRS (2j, 2j+1). Per chunk one
  [128,1024] PSUM tile holds both heads' scores; the two K=64 score matmuls
  are row-tiles at (0,0)/(64,0) and run concurrently on the PE. The two PV
  matmuls are M=64 col-tiles at (0,0)/(0,64), also concurrent.
- exp alternates strictly between ACT (exact, even chunks) and DVE (odd
  chunks) via a
  Schraudolph bit-trick: tensor_scalar(mult,add) f32->int16 with round-to-
  nearest lands the bf16 bit pattern of ~exp(score/8); the PE reads it as
  bf16 (probs tile bitcast). Strict alternation (incl. c15) avoids
  same-engine back-to-back exps that stall score-slot release. rel-err vs
  f32 reference 1.18e-2 (budget 2e-2).
- HW CONSTRAINT (measured, smoke2-4): concurrent matmul drains into a shared
  PSUM bank corrupt accumulation nondeterministically -- every accumulation
  region gets its OWN bank. attA [64,512] / attB rows 64: of [128,512] (one
  bank each, one pair in flight); denominators are NOT a V ones-column (would
  need M=65, breaking the col-tile pair) but a batched per-pair tail phase of
  ones-column N=512 matmuls over the retained probs tiles (den-A bank0 /
  den-B bank1 of one borrowed ps_sc slot; one LDWEIGHTS, back-to-back pairs
  at ~216ns).
- PSUM: 3x2-bank score slots (probs lookahead 2 chunks, decoupling the
  1.1-1.45us exp latency from the PE chain) + 2x1-bank att slots = 8 banks.
- Pair boundary: the next pair's scores(c0)/(c1) (and their exps) are
  emitted BEFORE the previous pair's den phase, so the exp pipeline refills
  during the den matmuls (boundary stall 5.6 -> 3.9us). The epilogue is
  split: bank-releasing att copies right after the den phase; the
  normalization chain (den-row staging to base-0 SBUF -- custom-DVE ops
  mis-read nonzero-base PSUM sources -- DVE reciprocal_approx_fast, GpSimd
  partition_broadcast (NOT gpsimd.tensor_mul: switching Q7 libraries costs a
  ~7us LIBRARY_RELOAD), DVE mul -> stack bf16) deferred past the next pair's
  first exps.
- Output projection: po alternates ps_sc slot / ps_at slot-pair (up to 5
  blocks in flight), p-outer matmul order, bf16 out, DMAs alternate queues.
"""
import math

import numpy as np
import ml_dtypes

import concourse.bacc as bacc
import concourse.mybir as mybir
import concourse.tile as tile
from concourse.bass_utils import run_bass_kernel_spmd

BF16 = mybir.dt.bfloat16
F32 = mybir.dt.float32
I16 = mybir.dt.int16
ADD = mybir.AluOpType.add
MULT = mybir.AluOpType.mult
EXP = mybir.ActivationFunctionType.Exp

B, S, D, H, DK = 2, 2048, 1024, 16, 64
HPC = 4            # heads per core
GD = HPC * DK      # head-group dim = 256
XW, QW, KW, VW = S, GD, GD, GD + 1    # record widths: 2048 | 256 | 256 | 257
REC = XW + QW + KW + VW               # 2817 (V: 4x64 packed dims + ones col)
OQ, OK, OV = XW, XW + QW, XW + QW + KW

SCALE = 1.0 / math.sqrt(DK)           # 1/8
A_SCH = SCALE * 128.0 / math.log(2.0)
B_SCH = 127.0 * 128.0 - 4.6           # round-to-nearest-optimal correction
QB = 512
NQB = S // QB                         # 4
# chunks whose exp runs on DVE (Schraudolph); rest on ACT (exact)
DVE_CHUNKS = frozenset((1, 3, 5, 7, 9, 11, 13, 15))
DBG_QB, DBG_HP = 1, 0

_NC_CACHE = {}


def build(kc=8, debug_dump=False):
    key = (kc, debug_dump)
    if key in _NC_CACHE:
        return _NC_CACHE[key]
    nc = bacc.Bacc("TRN2", target_bir_lowering=False, debug=False, num_devices=8)
    chunks = nc.declare_dram_parameter("chunks", [kc, 128, REC], BF16, isOutput=False)
    wo = nc.declare_dram_parameter("wo", [128, 2 * D + DK], BF16, isOutput=False)
    out = nc.declare_dram_parameter("out", [S, D], BF16, isOutput=True)
    if debug_dump:
        dbg_qt = nc.declare_dram_parameter("dbg_qt", [4, 128, S], BF16, isOutput=True)
        dbg_v = nc.declare_dram_parameter("dbg_v", [16, 128, VW], BF16, isOutput=True)
        dbg_probs = nc.declare_dram_parameter("dbg_probs", [4, 128, 1024], BF16, isOutput=True)
        dbg_att = nc.declare_dram_parameter("dbg_att", [2, 128, QB], F32, isOutput=True)
        dbg_stack = nc.declare_dram_parameter("dbg_stack", [2, 128, S], BF16, isOutput=True)
        dbg_rec = nc.declare_dram_parameter("dbg_rec", [2, QB], F32, isOutput=True)
        dbg_bc = nc.declare_dram_parameter("dbg_bc", [2, 64, QB], F32, isOutput=True)

    with tile.TileContext(nc) as tc:
        with (
            nc.allow_low_precision(reason="bf16 matmuls + int16 exp trick"),
            tc.tile_pool(name="wop", bufs=1) as wop,
            tc.tile_pool(name="qkp", bufs=4) as qkp,
            tc.tile_pool(name="vp", bufs=16) as vp,
            tc.tile_pool(name="stackp", bufs=2) as stackp,
            tc.tile_pool(name="recp", bufs=4) as recp,
            tc.tile_pool(name="bcp", bufs=4) as bcp,
            tc.tile_pool(name="probsp", bufs=19) as probsp,
            tc.tile_pool(name="outp", bufs=8) as outp,
            tc.tile_pool(name="ps_sc", bufs=3, space="PSUM") as ps_sc,   # 3x2 banks
            tc.tile_pool(name="ps_at", bufs=2, space="PSUM") as ps_at,   # 2x1 bank
        ):
            # ---- projections ----
            with tc.tile_pool(name="chp", bufs=kc) as chp:
                ch = []
                for c in range(kc):
                    t = chp.tile([128, REC], BF16, tag="ch", name=f"ch{c}")
                    # split each chunk across both DMA queues: halves the
                    # per-chunk latency so the first QK matmuls start sooner
                    nc.sync.dma_start(t[0:64, :], chunks[c][0:64, :])
                    nc.gpsimd.dma_start(t[64:128, :], chunks[c][64:128, :])
                    ch.append(t)
                wo_t = wop.tile([128, 2 * D + DK], BF16, tag="wo")
                nc.sync.dma_start(wo_t[:], wo[:])

                qt = [qkp.tile([128, S], BF16, tag="qk", name=f"qt{j}") for j in range(2)]
                kt = [qkp.tile([128, S], BF16, tag="qk", name=f"kt{j}") for j in range(2)]

                # Q^T/K^T, chunk-outer accumulation. PSUM budget: Q halves +
                # K half0 as 3x[128,1024] (ps_sc), K half1 as 2x[128,512]
                # (ps_at). Same-lhsT matmuls grouped -> one LDWEIGHTS per 4.
                def qk_group(j):
                    qaccs = [ps_sc.tile([128, 1024], F32, tag="sc",
                                        name=f"qacc{j}_{t2}") for t2 in range(2)]
                    kacc0 = ps_sc.tile([128, 1024], F32, tag="sc",
                                       name=f"kacc0_{j}")
                    kacc1 = [ps_at.tile([128, 512], F32, tag="at",
                                        name=f"kacc1_{j}_{q}") for q in range(2)]

                    def kslice(t2, q):
                        return kacc0[:, q * 512:(q + 1) * 512] if t2 == 0 \
                            else kacc1[q][:]

                    for c in range(kc):
                        for t2 in range(2):
                            for q in range(2):
                                nc.tensor.matmul(
                                    qaccs[t2][:, q * 512:(q + 1) * 512],
                                    ch[c][:, OQ + j * 128:OQ + j * 128 + 128],
                                    ch[c][:, t2 * 1024 + q * 512:
                                           t2 * 1024 + (q + 1) * 512],
                                    start=(c == 0), stop=(c == kc - 1),
                                )
                        for t2 in range(2):
                            for q in range(2):
                                nc.tensor.matmul(
                                    kslice(t2, q),
                                    ch[c][:, OK + j * 128:OK + j * 128 + 128],
                                    ch[c][:, t2 * 1024 + q * 512:
                                           t2 * 1024 + (q + 1) * 512],
                                    start=(c == 0), stop=(c == kc - 1),
                                )
                    for t2 in range(2):
                        nc.vector.tensor_copy(
                            qt[j][:, t2 * 1024:(t2 + 1) * 1024], qaccs[t2][:])
                    nc.scalar.copy(kt[j][:, 0:1024], kacc0[:])
                    for q in range(2):
                        nc.scalar.copy(
                            kt[j][:, 1024 + q * 512:1024 + (q + 1) * 512],
                            kacc1[q][:])

                qk_group(0)
                # absorb the wo DMA into PE's clock (1-wait rule)
                dmy = ps_at.tile([32, 32], F32, tag="at")
                nc.tensor.matmul(
                    dmy[:], wo_t[0:32, 0:32], wo_t[0:32, 32:64],
                    start=True, stop=True,
                )
                qk_group(1)

                v_sb = []
                for i in range(16):
                    acc = ps_at.tile([128, VW], F32, tag="at")
                    for c in range(kc):
                        nc.tensor.matmul(
                            acc[:], ch[c][:, i * 128:(i + 1) * 128],
                            ch[c][:, OV:OV + VW],
                            start=(c == 0), stop=(c == kc - 1),
                        )
                    vt = vp.tile([128, VW], BF16, tag="v", name=f"v{i}")
                    nc.vector.tensor_copy(vt[:], acc[:])
                    if kc == 8:
                        # no ones-row in x: set the denominator-matmul ones col
                        nc.vector.memset(vt[:, GD:GD + 1], 1.0)
                    v_sb.append(vt)

                if debug_dump:
                    for n, t in enumerate(qt + kt):
                        nc.sync.dma_start(dbg_qt[n], t[:])
                    for i in range(16):
                        nc.sync.dma_start(dbg_v[i], v_sb[i][:])

            # ---- attention: qb outer, head-pair inner ----
            stack = [stackp.tile([128, S], BF16, tag="stk", name=f"stack{j}")
                     for j in range(2)]

            def epilogue_banks(qb, hp, attA, attB):
                """Evacuate the pair's att accumulators to SBUF, releasing
                their PSUM banks for the next pair (DVE + ACT copies)."""
                aA = bcp.tile([64, QB], F32, tag="asb", name=f"aA{qb}_{hp}")
                nc.vector.tensor_copy(aA[:], attA[:])
                aB = bcp.tile([64, QB], F32, tag="asb", name=f"aB{qb}_{hp}")
                nc.scalar.copy(aB[:], attB[64:128, :])
                return aA, aB

            def epilogue_norm(qb, hp, aA, aB, den):
                """Normalize into stack. Deferred past the next pair's first
                exps so those don't queue behind these engine ops."""
                recs = []
                for hx, dsrc in ((0, den[0:1, 0:512]), (1, den[32:33, 512:1024])):
                    dn = recp.tile([1, QB], F32, tag="den",
                                   name=f"dn{qb}_{hp}_{hx}")
                    nc.scalar.copy(dn[:], dsrc)
                    rec = recp.tile([1, QB], F32, tag="rec",
                                    name=f"rec{qb}_{hp}_{hx}")
                    nc.vector.reciprocal_approx_fast(rec[:], dn[:])
                    recs.append(rec)
                for hx, asrc in ((0, aA), (1, aB)):
                    bc = bcp.tile([64, QB], F32, tag="bc",
                                  name=f"bc{qb}_{hp}_{hx}")
                    nc.gpsimd.partition_broadcast(bc[:], recs[hx][:])
                    if debug_dump and qb == DBG_QB and hp == DBG_HP:
                        nc.sync.dma_start(dbg_rec[hx:hx + 1, :], recs[hx][:])
                        nc.sync.dma_start(dbg_bc[hx], bc[:])
                    # NOT gpsimd.tensor_mul: switching Q7 libraries between
                    # partition_broadcast and tensor_tensor costs a ~7us
                    # LIBRARY_RELOAD per switch; ScalarE has no tensor_tensor.
                    nc.vector.tensor_mul(
                        stack[hp][hx * 64:(hx + 1) * 64, qb * QB:(qb + 1) * QB],
                        asrc[:],
                        bc[:],
                    )

            def scores_emit(qb, hp, c):
                """Row-tiled pair of score matmuls + exp -> probs [128,1024]
                (cols 0:512 head 2hp, 512:1024 head 2hp+1)."""
                sc = ps_sc.tile([128, 1024], F32, tag="sc")
                for hx in range(2):
                    rows = slice(hx * 64, (hx + 1) * 64)
                    nc.tensor.matmul(
                        sc[:, hx * 512:(hx + 1) * 512],
                        kt[hp][rows, c * 128:(c + 1) * 128],
                        qt[hp][rows, qb * QB:(qb + 1) * QB],
                        start=True, stop=True,
                    )
                probs = probsp.tile([128, 1024], BF16, tag="probs",
                                    name=f"probs{qb}_{hp}_{c}")
                if c in DVE_CHUNKS:
                    nc.vector.tensor_scalar(
                        probs[:].bitcast(I16), sc[:], A_SCH, B_SCH, MULT, ADD)
                else:
                    nc.scalar.activation(probs[:], sc[:], EXP, scale=SCALE)
                return probs

            def den_phase(qb, hp, probs_all):
                """Batched denominator matmuls over the retained probs: one
                borrowed ps_sc slot, den-A bank 0 / den-B bank 1, back-to-back
                concurrent N=512 ones-column matmuls."""
                den = ps_sc.tile([33, 1024], F32, tag="sc",
                                 name=f"den{qb}_{hp}")
                for c in range(16):
                    nc.tensor.matmul(
                        den[0:1, 0:512], v_sb[c][:, GD:GD + 1],
                        probs_all[c][:, 0:512],
                        start=(c == 0), stop=(c == 15),
                    )
                    nc.tensor.matmul(
                        den[32:33, 512:1024], v_sb[c][:, GD:GD + 1],
                        probs_all[c][:, 512:1024],
                        start=(c == 0), stop=(c == 15),
                        skip_group_check=True,
                    )
                return den

            prev = None
            for qb in range(NQB):
                for hp in range(2):
                    attA = ps_at.tile([64, QB], F32, tag="at",
                                      name=f"attA{qb}_{hp}")
                    attB = ps_at.tile([128, QB], F32, tag="at",
                                      name=f"attB{qb}_{hp}")
                    probs_all = []
                    # emit this pair's first scores+exps BEFORE the previous
                    # pair's den phase: the exps run on ACT/DVE during the
                    # den matmuls, so the chunk loop restarts with a full
                    # pipeline instead of paying a ~5us refill ramp.
                    probs_q = [scores_emit(qb, hp, 0), scores_emit(qb, hp, 1)]
                    norm_pending = None
                    if prev is not None:
                        pqb, php, pattA, pattB, pprobs = prev
                        pden = den_phase(pqb, php, pprobs)
                        paA, paB = epilogue_banks(pqb, php, pattA, pattB)
                        norm_pending = (pqb, php, paA, paB, pden)
                        prev = None

                    for c in range(16):
                        if c + 2 < 16:
                            probs_q.append(scores_emit(qb, hp, c + 2))
                        if c == 2 and norm_pending is not None:
                            epilogue_norm(*norm_pending)
                            norm_pending = None
                        probs = probs_q.pop(0)
                        probs_all.append(probs)
                        if debug_dump and qb == DBG_QB and hp == DBG_HP and c < 4:
                            nc.sync.dma_start(dbg_probs[c], probs[:])
                        nc.tensor.matmul(
                            attA[:], v_sb[c][:, 2 * hp * 64:(2 * hp + 1) * 64],
                            probs[:, 0:512],
                            start=(c == 0), stop=(c == 15),
                        )
                        nc.tensor.matmul(
                            attB[64:128, :],
                            v_sb[c][:, (2 * hp + 1) * 64:(2 * hp + 2) * 64],
                            probs[:, 512:1024],
                            start=(c == 0), stop=(c == 15),
                            skip_group_check=True,
                        )
                    prev = (qb, hp, attA, attB, probs_all)
                    if debug_dump and qb == DBG_QB and hp == DBG_HP:
                        den = den_phase(qb, hp, probs_all)
                        aA_, aB_ = epilogue_banks(qb, hp, attA, attB)
                        epilogue_norm(qb, hp, aA_, aB_, den)
                        prev = None
                        asb = bcp.tile([128, QB], F32, tag="asbdbg",
                                       name="asbdbg0")
                        nc.vector.tensor_copy(asb[0:64, :], attA[:])
                        nc.vector.tensor_copy(asb[64:128, :], attB[64:128, :])
                        nc.sync.dma_start(dbg_att[0], asb[:])
                        dsb = bcp.tile([33, QB], F32, tag="asbdbg",
                                       name="asbdbg1")
                        nc.vector.tensor_copy(dsb[0:1, :], den[0:1, 0:512])
                        nc.vector.tensor_copy(dsb[32:33, :], den[32:33, 512:1024])
                        nc.sync.dma_start(dbg_att[1][0:33], dsb[:])
            if prev is not None:
                pqb, php, pattA, pattB, pprobs = prev
                pden = den_phase(pqb, php, pprobs)
                paA, paB = epilogue_banks(pqb, php, pattA, pattB)
                epilogue_norm(pqb, php, paA, paB, pden)
            if debug_dump:
                for p in range(2):
                    nc.sync.dma_start(dbg_stack[p], stack[p][:])

            # ---- output projection (bf16 out) ----
            # po alternates between ps_sc slots (2 banks) and ps_at slot
            # pairs (1 bank each) -> up to 5 blocks in flight so the evac
            # copies stay off the critical path. p-outer order reuses each
            # loaded stack stationary for 2 MMs.
            for i in range(16):
                osb = outp.tile([128, 1024], BF16, tag="out", name="osb")
                if i % 2 == 0:
                    po2 = ps_sc.tile([128, 1024], F32, tag="sc", name=f"po{i}")
                    po = [po2[:, 0:512], po2[:, 512:1024]]
                else:
                    po = [ps_at.tile([128, 512], F32, tag="at",
                                     name=f"po{i}_{n}")[:] for n in range(2)]
                for p in range(2):
                    for n in range(2):
                        nc.tensor.matmul(
                            po[n],
                            stack[p][:, i * 128:(i + 1) * 128],
                            wo_t[:, p * D + n * 512:p * D + (n + 1) * 512],
                            start=(p == 0), stop=(p == 1),
                        )
                nc.vector.tensor_copy(osb[:, 0:512], po[0])
                nc.scalar.copy(osb[:, 512:1024], po[1])
                eng = nc.gpsimd if i % 2 == 0 else nc.sync
                eng.dma_start(out[i * 128:(i + 1) * 128, :], osb[:])

    nc.compile()
    _NC_CACHE[kc] = nc
    return nc


def make_core_inputs(x, W_q, b_q, W_k, b_k, W_v, b_v, W_o):
    """Host-side shard + layout prep for core (b, g). Returns (ins, kc)."""
    use_bias = any(np.any(np.asarray(b)) for b in (b_q, b_k, b_v))
    kc = 9 if use_bias else 8
    krows = kc * 128
    ins = []
    for core in range(8):
        b, g = core // 4, core % 4
        sl = slice(g * GD, (g + 1) * GD)

        xa = np.zeros((krows, S), np.float32)
        xa[:D] = np.asarray(x[b]).T

        qa = np.zeros((krows, QW), np.float32)
        qa[:D] = np.asarray(W_q[sl]).T
        ka = np.zeros((krows, KW), np.float32)
        ka[:D] = np.asarray(W_k[sl]).T

        va = np.zeros((krows, VW), np.float32)
        va[:D, :GD] = np.asarray(W_v[sl]).T  # [1024, 256] packed V dims

        if use_bias:
            xa[D] = 1.0
            qa[D] = np.asarray(b_q[sl])
            ka[D] = np.asarray(b_k[sl])
            va[D, :GD] = np.asarray(b_v[sl])
            va[D, GD] = 1.0

        chunks = np.concatenate([xa, qa, ka, va], axis=1).reshape(kc, 128, REC)

        wo = np.zeros((128, 2 * D + DK), np.float32)
        wot = np.asarray(W_o[:, sl]).T  # [256, 1024] = W_o^T rows for group g
        wo[:, :D] = wot[:128]
        wo[:, D:2 * D] = wot[128:]
        wo[:, 2 * D:] = 1.0

        ins.append({
            "chunks": np.ascontiguousarray(chunks.astype(ml_dtypes.bfloat16)),
            "wo": np.ascontiguousarray(wo.astype(ml_dtypes.bfloat16)),
        })
    return ins, kc


def run_cores(ins, kc=8, trace=False, tmpdir=None, debug_dump=False):
    nc = build(kc=kc, debug_dump=debug_dump)
    return run_bass_kernel_spmd(nc, ins, list(range(8)), trace=trace,
                                tmpdir=tmpdir)


def kernel(x, attention_mask, W_q, b_q, W_k, b_k, W_v, b_v, W_o, b_o, _trace=False,
           _res_out=None, _tmpdir=None):
    # attention_mask is all-ones for this problem (spec fill=ones): the
    # reference's masking is a no-op, so it is not applied on device.
    ins, kc = make_core_inputs(x, W_q, b_q, W_k, b_k, W_v, b_v, W_o)
    res = run_cores(ins, kc=kc, trace=_trace, tmpdir=_tmpdir)
    if _res_out is not None:
        _res_out.append(res)
    bo = np.asarray(b_o, np.float32)
    out = np.empty((B, S, D), np.float32)
    for b in range(B):
        acc = res.results[4 * b]["out"].astype(np.float32).copy()
        for g in range(1, 4):
            acc += res.results[4 * b + g]["out"].astype(np.float32)
        out[b] = acc + bo
    return out
